# revision 1
# baseline (speedup 1.0000x reference)
"""Decoder block (pre-norm attention + FFN) on 8 TRN2 NeuronCores.

Sharding: each batch element (B=4) is owned by 2 cores. Within a batch pair,
queries are split causally-balanced: core j=0 takes q-blocks {0,3} (4+16
kc-chunks of valid keys), j=1 takes {1,2} (8+12). To keep the program
uniform across cores (one NEFF), every core runs two fixed-size attention
"jobs": job0 = 8 kc-chunks, job1 = 16 kc-chunks; causal validity (and the
4-kc / 12-kc occupants' unused chunks) is enforced by host-supplied
multiplicative bf16 masks applied to exp(S). All matmuls run in float32r
(tf32-like) for full-rate PE with ~1e-4 relative error; the residual
stream stays float32.

Layout is "transposed": activations are [d_model, tokens] so every matmul
contracts over partitions. LayerNorm stats are computed with ones-column
matmuls (partition reduction on PE) and broadcast back via K=1 matmuls
into PSUM. Softmax row-sums come from an appended ones-column in V
(out row 64 of each AV matmul); normalization uses a reciprocal row
broadcast via gpsimd.partition_broadcast.
"""

import numpy as np
import ml_dtypes

B, T, D = 4, 2048, 768
H, DK, DFF = 12, 64, 3072
DC = D // 128        # 6 chunks of d_model
FC = DFF // 128      # 24 chunks of d_ff
NFF = DFF // 512     # 6 ff column groups
EPS = 1e-5
NCORES = 8
QCOLS = 1024         # q columns per core
JOB_KC = (8, 16)
NMASK = JOB_KC[0] + JOB_KC[1]
ABLK = 256           # column block for LN/KV streaming
QB_ASSIGN = {0: (0, 3), 1: (1, 2)}  # core j -> (job0 qb, job1 qb), 512-row blocks

_cache = {}


def _build(apply_ln1, apply_ln2):
    import concourse.bacc as bacc
    import concourse.tile as tile
    import concourse.mybir as mybir
    from contextlib import ExitStack

    dt = mybir.dt
    F = mybir.ActivationFunctionType
    OP = mybir.AluOpType

    nc = bacc.Bacc("TRN2", target_bir_lowering=False, debug=False)

    # ---- DRAM I/O ----
    xtf = nc.dram_tensor("xtf", [128, DC, T], dt.float32, kind="ExternalInput")
    xtq = nc.dram_tensor("xtq", [128, DC, QCOLS], dt.float32, kind="ExternalInput")
    wq = nc.dram_tensor("wq", [128, DC, D], dt.float32r, kind="ExternalInput")
    wk = nc.dram_tensor("wk", [128, DC, D], dt.float32r, kind="ExternalInput")
    wv = nc.dram_tensor("wv", [128, DC, D], dt.float32r, kind="ExternalInput")
    wo = nc.dram_tensor("wo", [128, DC, D], dt.float32r, kind="ExternalInput")
    w1 = nc.dram_tensor("w1", [128, DC, DFF], dt.float32r, kind="ExternalInput")
    w2 = nc.dram_tensor("w2", [128, FC, D], dt.float32r, kind="ExternalInput")
    bq_d = nc.dram_tensor("bq", [128, DC], dt.float32, kind="ExternalInput")
    bk_d = nc.dram_tensor("bk", [128, DC], dt.float32, kind="ExternalInput")
    bv_d = nc.dram_tensor("bv", [1, D], dt.float32r, kind="ExternalInput")
    bo_d = nc.dram_tensor("bo", [128, DC], dt.float32, kind="ExternalInput")
    b1_d = nc.dram_tensor("b1", [128, FC], dt.float32, kind="ExternalInput")
    b2_d = nc.dram_tensor("b2", [128, DC], dt.float32, kind="ExternalInput")
    masks_d = nc.dram_tensor("masks", [128, NMASK, 512], dt.bfloat16, kind="ExternalInput")
    vones_d = nc.dram_tensor("vones", [128, T // 128, H, 1], dt.float32r, kind="ExternalInput")
    if apply_ln1:
        g1_d = nc.dram_tensor("g1", [128, DC], dt.float32, kind="ExternalInput")
        c1_d = nc.dram_tensor("c1", [128, DC], dt.float32, kind="ExternalInput")
    if apply_ln2:
        g2_d = nc.dram_tensor("g2", [128, DC], dt.float32, kind="ExternalInput")
        c2_d = nc.dram_tensor("c2", [128, DC], dt.float32, kind="ExternalInput")
    y_d = nc.dram_tensor("y", [128, DC, QCOLS], dt.float32, kind="ExternalOutput")

    NKC = T // 128  # 16 key chunks

    with tile.TileContext(nc) as tc, ExitStack() as ctx:
        const = ctx.enter_context(tc.tile_pool(name="const", bufs=1))
        ones_f = const.tile([128, 1], dt.float32)
        nc.vector.memset(ones_f[:], 1.0)
        ones_col = const.tile([128, 1], dt.float32r)       # stats lhsT [K=128, M=1]
        nc.vector.tensor_copy(ones_col[:], ones_f[:])
        ones_row = const.tile([1, 128], dt.float32r)       # K=1 bcast lhsT [K=1, M=128]
        nc.vector.tensor_copy(ones_row[:], ones_f[0:1, :].to_broadcast([1, 128]))
        eps_t = const.tile([1, 1], dt.float32)
        nc.vector.memset(eps_t[:], EPS)
        bq_t = const.tile([128, DC], dt.float32)
        bk_t = const.tile([128, DC], dt.float32)
        bo_t = const.tile([128, DC], dt.float32)
        b1_t = const.tile([128, FC], dt.float32)
        b2_t = const.tile([128, DC], dt.float32)
        bv_t = const.tile([1, D], dt.float32r)
        nc.sync.dma_start(bq_t[:], bq_d[:])
        nc.sync.dma_start(bk_t[:], bk_d[:])
        nc.sync.dma_start(bo_t[:], bo_d[:])
        nc.sync.dma_start(b1_t[:], b1_d[:])
        nc.sync.dma_start(b2_t[:], b2_d[:])
        nc.sync.dma_start(bv_t[:], bv_d[:])
        if apply_ln1:
            g1_t = const.tile([128, DC], dt.float32)
            c1_t = const.tile([128, DC], dt.float32)
            nc.sync.dma_start(g1_t[:], g1_d[:])
            nc.sync.dma_start(c1_t[:], c1_d[:])
        if apply_ln2:
            g2_t = const.tile([128, DC], dt.float32)
            c2_t = const.tile([128, DC], dt.float32)
            nc.sync.dma_start(g2_t[:], g2_d[:])
            nc.sync.dma_start(c2_t[:], c2_d[:])

        def layernorm_block(xt_cb, ht_cb, ncols, pool, psum, apply_affine, g_t, c_t, rpool=None):
            """LN over partitions (d_model) of xt_cb [128, DC, ncols] -> ht_cb f32r."""
            s_ps = psum.tile([1, ncols], dt.float32, tag="lnS")
            q_ps = psum.tile([1, ncols], dt.float32, tag="lnQ")
            for c in range(DC):
                xr = pool.tile([128, ncols], dt.float32r, tag="lnxr")
                sq = pool.tile([128, ncols], dt.float32r, tag="lnsq")
                nc.vector.tensor_copy(xr[:], xt_cb[:, c, :])
                nc.scalar.activation(out=sq[:], in_=xt_cb[:, c, :], func=F.Square)
                nc.tensor.matmul(s_ps[:], ones_col[:], xr[:],
                                 start=(c == 0), stop=(c == DC - 1))
                nc.tensor.matmul(q_ps[:], ones_col[:], sq[:],
                                 start=(c == 0), stop=(c == DC - 1))
            rp = rpool or pool
            mu = rp.tile([1, ncols], dt.float32, tag="lnmu")
            msq = rp.tile([1, ncols], dt.float32, tag="lnmsq")
            nc.vector.tensor_scalar_mul(mu[:], s_ps[:], 1.0 / D)
            nc.vector.tensor_scalar_mul(msq[:], q_ps[:], 1.0 / D)
            var = rp.tile([1, ncols], dt.float32, tag="lnvar")
            nc.vector.tensor_mul(var[:], mu[:], mu[:])
            nc.vector.tensor_sub(var[:], msq[:], var[:])
            std = rp.tile([1, ncols], dt.float32, tag="lnstd")
            nc.scalar.activation(out=std[:], in_=var[:], func=F.Sqrt, bias=eps_t[:])
            rstd = rp.tile([1, ncols], dt.float32, tag="lnrstd")
            nc.vector.reciprocal(rstd[:], std[:])
            rstd_r = rp.tile([1, ncols], dt.float32r, tag="lnrstdr")
            nc.vector.tensor_copy(rstd_r[:], rstd[:])
            nmu_r = rp.tile([1, ncols], dt.float32r, tag="lnnmur")
            nc.vector.tensor_scalar_mul(nmu_r[:], mu[:], -1.0)
            nmu_ps = psum.tile([128, ncols], dt.float32, tag="lnbc1")
            rstd_ps = psum.tile([128, ncols], dt.float32, tag="lnbc2")
            nc.tensor.matmul(nmu_ps[:], ones_row[:], nmu_r[:], start=True, stop=True)
            nc.tensor.matmul(rstd_ps[:], ones_row[:], rstd_r[:], start=True, stop=True)
            for c in range(DC):
                t1 = pool.tile([128, ncols], dt.float32, tag="lnt1")
                nc.vector.tensor_add(t1[:], xt_cb[:, c, :], nmu_ps[:])
                if apply_affine:
                    t2 = pool.tile([128, ncols], dt.float32, tag="lnt2")
                    nc.vector.tensor_mul(t2[:], t1[:], rstd_ps[:])
                    nc.vector.tensor_scalar(
                        out=ht_cb[:, c, :], in0=t2[:],
                        scalar1=g_t[:, c:c + 1], scalar2=c_t[:, c:c + 1],
                        op0=OP.mult, op1=OP.add)
                else:
                    nc.vector.tensor_mul(ht_cb[:, c, :], t1[:], rstd_ps[:])

        pKVQ = tc.alloc_tile_pool(name="kvq", bufs=1)
        KT = pKVQ.tile([128, DC, T], dt.float32r)
        VA = pKVQ.tile([128, NKC, H, 65], dt.float32r)
        QT = pKVQ.tile([128, DC, QCOLS], dt.float32r)
        nc.sync.dma_start(VA[:, :, :, 64:65], vones_d[:])

        # ---- Phase AB-kv: stream x, LN1, project K and V ----
        with (
            tc.tile_pool(name="abkv", bufs=2) as pab,
            tc.tile_pool(name="abw", bufs=1) as pw,
            tc.tile_pool(name="abps", bufs=1) as psln,
            tc.tile_pool(name="abps2", bufs=2, space="PSUM") as psp,
            tc.tile_pool(name="ablps", bufs=1, space="PSUM") as psl,
        ):
            wk_t = pw.tile([128, DC, D], dt.float32r, tag="wkv")
            wv_t = pw.tile([128, DC, D], dt.float32r, tag="wkv2")
            nc.sync.dma_start(wk_t[:], wk[:])
            nc.sync.dma_start(wv_t[:], wv[:])
            for cb in range(T // ABLK):
                cs = slice(cb * ABLK, (cb + 1) * ABLK)
                xt_cb = pab.tile([128, DC, ABLK], dt.float32, tag="xt")
                nc.sync.dma_start(xt_cb[:], xtf[:, :, cs])
                ht_cb = pab.tile([128, DC, ABLK], dt.float32r, tag="ht")
                layernorm_block(xt_cb, ht_cb, ABLK, pab, psl,
                                apply_ln1, g1_t if apply_ln1 else None,
                                c1_t if apply_ln1 else None)
                # K projection: KT[:, m, cs] = (h @ Wk + bk)^T
                for m in range(DC):
                    kps = psp.tile([128, ABLK], dt.float32, tag="kps")
                    for c in range(DC):
                        nc.tensor.matmul(kps[:], wk_t[:, c, m * 128:(m + 1) * 128],
                                         ht_cb[:, c, :], start=(c == 0), stop=(c == DC - 1))
                    nc.scalar.activation(out=KT[:, m, cs], in_=kps[:], func=F.Identity,
                                         bias=bk_t[:, m:m + 1])
                # V projection (row-major + bias via ones-row matmul)
                for rc in range(ABLK // 128):
                    rs = slice(cb * ABLK + rc * 128, cb * ABLK + rc * 128 + 128)
                    kc_idx = cb * (ABLK // 128) + rc
                    for nh in range(2):
                        ns = slice(nh * 384, (nh + 1) * 384)
                        vps = psp.tile([128, 384], dt.float32, tag="vps")
                        for c in range(DC):
                            nc.tensor.matmul(vps[:], ht_cb[:, c, rc * 128:(rc + 1) * 128],
                                             wv_t[:, c, ns], start=(c == 0), stop=False)
                        nc.tensor.matmul(vps[:], ones_row[:], bv_t[:, ns],
                                         start=False, stop=True)
                        nc.scalar.activation(
                            out=VA[:, kc_idx, nh * 6:(nh + 1) * 6, 0:64],
                            in_=vps[:].rearrange("p (h d) -> p h d", d=64),
                            func=F.Copy)

        # ---- Phase AB-q: stream xq, LN1, project Q ----
        with (
            tc.tile_pool(name="abq", bufs=2) as pab,
            tc.tile_pool(name="abqw", bufs=1) as pw,
            tc.tile_pool(name="abqps", bufs=2, space="PSUM") as psp,
            tc.tile_pool(name="abqlps", bufs=1, space="PSUM") as psl,
        ):
            wq_t = pw.tile([128, DC, D], dt.float32r)
            nc.sync.dma_start(wq_t[:], wq[:])
            for cb in range(QCOLS // ABLK):
                cs = slice(cb * ABLK, (cb + 1) * ABLK)
                xt_cb = pab.tile([128, DC, ABLK], dt.float32, tag="xt")
                nc.sync.dma_start(xt_cb[:], xtq[:, :, cs])
                ht_cb = pab.tile([128, DC, ABLK], dt.float32r, tag="ht")
                layernorm_block(xt_cb, ht_cb, ABLK, pab, psl,
                                apply_ln1, g1_t if apply_ln1 else None,
                                c1_t if apply_ln1 else None)
                for m in range(DC):
                    qps = psp.tile([128, ABLK], dt.float32, tag="qps")
                    for c in range(DC):
                        nc.tensor.matmul(qps[:], wq_t[:, c, m * 128:(m + 1) * 128],
                                         ht_cb[:, c, :], start=(c == 0), stop=(c == DC - 1))
                    nc.scalar.activation(out=QT[:, m, cs], in_=qps[:], func=F.Identity,
                                         bias=bq_t[:, m:m + 1])

        # ---- Phase C: attention (attnT stays in SBUF) ----
        pATT = tc.alloc_tile_pool(name="attp", bufs=1, side="right")
        AT = pATT.tile([128, DC, QCOLS], dt.float32r)
        with (
            tc.tile_pool(name="cmask", bufs=1) as pm,
            tc.tile_pool(name="cloc", bufs=3) as pc,
            tc.tile_pool(name="cloc2", bufs=2) as pc2,
            tc.tile_pool(name="cps", bufs=2, space="PSUM") as psc,
            tc.tile_pool(name="cps2", bufs=2, space="PSUM") as pso,
        ):
            mk_t = pm.tile([128, NMASK, 512], dt.bfloat16)
            nc.sync.dma_start(mk_t[:], masks_d[:])
            tidx = 0
            for jb in range(2):
                qs = slice(jb * 512, (jb + 1) * 512)
                midx0 = 0 if jb == 0 else JOB_KC[0]
                for m in range(DC):          # head pair (2m, 2m+1)
                    out2e = pso.tile([128, 512], dt.float32, tag="out2e")
                    out2o = pso.tile([128, 512], dt.float32, tag="out2o")
                    for kc in range(JOB_KC[jb]):
                        sps = psc.tile([128, 2, 512], dt.float32, tag="sps")
                        # back-to-back S matmuls on row-groups 0-1 / 2-3
                        nc.tensor.matmul(sps[:, 0, :], KT[0:64, m, kc * 128:(kc + 1) * 128],
                                         QT[0:64, m, qs], start=True, stop=True)
                        nc.tensor.matmul(sps[:, 1, :], KT[64:128, m, kc * 128:(kc + 1) * 128],
                                         QT[64:128, m, qs], start=True, stop=True)
                        pmm = pc.tile([128, 2, 512], dt.float32r, tag="pmm")
                        nc.scalar.activation(out=pmm[:], in_=sps[:], func=F.Exp)
                        for par in range(2):
                            eng = nc.gpsimd if tidx % 3 == 2 else nc.vector
                            eng.tensor_mul(pmm[:, par, :], pmm[:, par, :],
                                           mk_t[:, midx0 + kc, :])
                            tidx += 1
                        for par, out2 in ((0, out2e), (1, out2o)):
                            nc.tensor.matmul(out2[0:65, :], VA[:, kc, 2 * m + par, :],
                                             pmm[:, par, :],
                                             start=(kc == 0), stop=(kc == JOB_KC[jb] - 1))
                    for par, out2 in ((0, out2e), (1, out2o)):
                        p0 = par * 64
                        rec = pc2.tile([128, 512], dt.float32, tag="rec")
                        nc.vector.reciprocal(rec[64:65, :], out2[64:65, :])
                        rec0 = pc2.tile([1, 512], dt.float32, tag="rec0")
                        nc.sync.dma_start(rec0[0:1, :], rec[64:65, :])
                        rrep = pc2.tile([64, 512], dt.float32, tag="rrep")
                        nc.gpsimd.partition_broadcast(rrep[:], rec0[0:1, :])
                        if par == 0:
                            nc.vector.tensor_mul(AT[0:64, m, qs], out2[0:64, :], rrep[:])
                        else:
                            sh = pc2.tile([64, 512], dt.float32r, tag="sh")
                            nc.vector.tensor_mul(sh[:], out2[0:64, :], rrep[:])
                            nc.sync.dma_start(AT[64:128, m, qs], sh[:])

        pKVQ.release()
        pX2 = ctx.enter_context(tc.tile_pool(name="x2p", bufs=1))
        x2 = pX2.tile([128, DC, QCOLS], dt.float32)

        # ---- Phase D: Wo + residual (attnT direct from SBUF) ----
        with (
            tc.tile_pool(name="dloc", bufs=2) as pd,
            tc.tile_pool(name="dw", bufs=1) as pwo,
            tc.tile_pool(name="dps", bufs=2, space="PSUM") as psd,
        ):
            wo_t = pwo.tile([128, DC, D], dt.float32r)
            nc.sync.dma_start(wo_t[:], wo[:])
            for cb in range(2):
                cs = slice(cb * 512, (cb + 1) * 512)
                xq_cb = pd.tile([128, DC, 512], dt.float32, tag="xqcb")
                nc.sync.dma_start(xq_cb[:], xtq[:, :, cs])
                for m in range(DC):
                    ops = psd.tile([128, 512], dt.float32, tag="ops")
                    for c in range(DC):
                        nc.tensor.matmul(ops[:], wo_t[:, c, m * 128:(m + 1) * 128],
                                         AT[:, c, cs], start=(c == 0), stop=(c == DC - 1))
                    nc.vector.scalar_tensor_tensor(
                        out=x2[:, m, cs], in0=ops[:], scalar=bo_t[:, m:m + 1],
                        in1=xq_cb[:, m, :], op0=OP.add, op1=OP.add)
        pATT.release()

        # ---- Phases E/F: LN2 + FFN ----
        with tc.tile_pool(name="dx", bufs=1) as px:
            h2 = px.tile([128, DC, QCOLS], dt.float32r)
            with (
                tc.tile_pool(name="eloc", bufs=2) as pe,
                tc.tile_pool(name="erows", bufs=1) as per,
                tc.tile_pool(name="elps", bufs=1, space="PSUM") as psl2,
                tc.tile_pool(name="fw", bufs=2) as pf,
                tc.tile_pool(name="fa", bufs=1) as pa,
                tc.tile_pool(name="fps", bufs=2, space="PSUM") as psa,
                tc.tile_pool(name="fps2", bufs=2, space="PSUM") as psy,
            ):
                for cb in range(2):
                    cs = slice(cb * 512, (cb + 1) * 512)
                    layernorm_block(x2[:, :, cs], h2[:, :, cs], 512, pe, psl2,
                                    apply_ln2, g2_t if apply_ln2 else None,
                                    c2_t if apply_ln2 else None, rpool=per)
                for c in range(DC):
                    nc.vector.tensor_scalar_add(x2[:, c, :], x2[:, c, :], b2_t[:, c:c + 1])
                for fp in range(NFF // 2):
                    w1f = pf.tile([128, DC, 1024], dt.float32r, tag="w1f")
                    w2f = pf.tile([128, 8, D], dt.float32r, tag="w2f")
                    nc.sync.dma_start(w1f[:], w1[:, :, fp * 1024:(fp + 1) * 1024])
                    nc.sync.dma_start(w2f[:], w2[:, fp * 8:(fp + 1) * 8, :])
                    for cb in range(2):
                        cs = slice(cb * 512, (cb + 1) * 512)
                        a1 = pa.tile([128, 8, 512], dt.float32r, tag="a1")
                        for mf in range(8):
                            aps = psa.tile([128, 512], dt.float32, tag="aps")
                            for c in range(DC):
                                nc.tensor.matmul(aps[:], w1f[:, c, mf * 128:(mf + 1) * 128],
                                                 h2[:, c, cs], start=(c == 0), stop=(c == DC - 1))
                            nc.scalar.activation(out=a1[:, mf, :], in_=aps[:], func=F.Gelu,
                                                 bias=b1_t[:, fp * 8 + mf:fp * 8 + mf + 1])
                        for m in range(DC):
                            yps = psy.tile([128, 512], dt.float32, tag="yps")
                            for mf in range(8):
                                nc.tensor.matmul(yps[:], w2f[:, mf, m * 128:(m + 1) * 128],
                                                 a1[:, mf, :], start=(mf == 0), stop=(mf == 7))
                            nc.vector.tensor_add(x2[:, m, cs], x2[:, m, cs], yps[:])
            nc.sync.dma_start(y_d[:], x2[:])

    nc.compile()
    return nc


def _to_lhsT(w):
    """[Din, Dout] -> [128, Din//128, Dout] (partition-chunked lhsT layout)."""
    din, dout = w.shape
    return np.ascontiguousarray(w.reshape(din // 128, 128, dout).transpose(1, 0, 2))


def _to_cols(b):
    """[D] -> [128, D//128]."""
    return np.ascontiguousarray(b.reshape(-1, 128).T)


def _to_tposed(xb):
    """[T?, 768] -> [128, 6, T?] transposed chunked layout."""
    t = xb.shape[0]
    return np.ascontiguousarray(xb.T.reshape(DC, 128, t).transpose(1, 0, 2))


def kernel(**inputs):
    from concourse.bass_utils import run_bass_kernel_spmd

    x = np.asarray(inputs["x"], np.float32)
    ln1_g = np.asarray(inputs["ln1_g"], np.float32)
    ln1_b = np.asarray(inputs["ln1_b"], np.float32)
    ln2_g = np.asarray(inputs["ln2_g"], np.float32)
    ln2_b = np.asarray(inputs["ln2_b"], np.float32)
    Wq = np.asarray(inputs["Wq"], np.float32) / np.sqrt(np.float32(DK))
    bq = np.asarray(inputs["bq"], np.float32) / np.sqrt(np.float32(DK))
    Wk = np.asarray(inputs["Wk"], np.float32)
    bk = np.asarray(inputs["bk"], np.float32)
    Wv = np.asarray(inputs["Wv"], np.float32)
    bv = np.asarray(inputs["bv"], np.float32)
    Wo = np.asarray(inputs["Wo"], np.float32)
    bo = np.asarray(inputs["bo"], np.float32)
    W1 = np.asarray(inputs["W1"], np.float32)
    b1 = np.asarray(inputs["b1"], np.float32)
    W2 = np.asarray(inputs["W2"], np.float32)
    b2 = np.asarray(inputs["b2"], np.float32)

    apply_ln1 = not (np.all(ln1_g == 1.0) and np.all(ln1_b == 0.0))
    apply_ln2 = not (np.all(ln2_g == 1.0) and np.all(ln2_b == 0.0))

    key = (apply_ln1, apply_ln2)
    if key not in _cache:
        _cache[key] = _build(apply_ln1, apply_ln2)
    nc = _cache[key]

    shared = {
        "wq": _to_lhsT(Wq), "wk": _to_lhsT(Wk), "wv": _to_lhsT(Wv),
        "wo": _to_lhsT(Wo), "w1": _to_lhsT(W1), "w2": _to_lhsT(W2),
        "bq": _to_cols(bq), "bk": _to_cols(bk), "bo": _to_cols(bo),
        "b1": _to_cols(b1), "b2": _to_cols(b2),
        "bv": bv.reshape(1, D).copy(),
        "vones": np.ones((128, T // 128, H, 1), np.float32),
    }
    if apply_ln1:
        shared["g1"] = _to_cols(ln1_g)
        shared["c1"] = _to_cols(ln1_b)
    if apply_ln2:
        shared["g2"] = _to_cols(ln2_g)
        shared["c2"] = _to_cols(ln2_b)

    in_maps = []
    qcols_per_core = []
    for core in range(NCORES):
        b, j = core // 2, core % 2
        qa, qb = QB_ASSIGN[j]
        qpos = np.concatenate([
            np.arange(qa * 512, qa * 512 + 512),
            np.arange(qb * 512, qb * 512 + 512),
        ])
        qcols_per_core.append(qpos)
        masks = np.zeros((128, NMASK, 512), np.float32)
        p = np.arange(128)[:, None]
        for kc in range(JOB_KC[0]):
            masks[:, kc, :] = (qpos[None, :512] >= kc * 128 + p)
        for kc in range(JOB_KC[1]):
            masks[:, JOB_KC[0] + kc, :] = (qpos[None, 512:] >= kc * 128 + p)
        m = dict(shared)
        m["xtf"] = _to_tposed(x[b])
        m["xtq"] = _to_tposed(x[b][qpos])
        m["masks"] = masks.astype(ml_dtypes.bfloat16)
        in_maps.append(m)

    res = run_bass_kernel_spmd(nc, in_maps, core_ids=list(range(NCORES)))

    y = np.empty((B, T, D), np.float32)
    for core in range(NCORES):
        b = core // 2
        yt = res.results[core]["y"]                       # [128, DC, QCOLS]
        y[b, qcols_per_core[core]] = yt.transpose(1, 0, 2).reshape(D, QCOLS).T
    return y

